# revision 29
# baseline (speedup 1.0000x reference)
"""GCNConv-style GNN layer on 8 Trainium2 NeuronCores (Bass/Tile).

Reference computation (B=8, N=4096, C=128, E=131072):
    adj  = symmetric 0/1 adjacency from edge_index, zero diagonal
    h    = x @ W0 + b0
    agg  = adj @ h            (per batch)
    out  = (cat[x, agg] @ W1 + b1) @ W2 + b2
    out  = gelu(out) @ Wo + bo
    ret  = x + out

Algebraic refactor (fold all linear maps before the single GELU on the
host at O(C^2) cost):
    W12  = W1 @ W2                  [2C, C]
    Wx   = W12[:C]                  x-path weight
    Wa   = W0 @ W12[C:]             agg-path weight applied to s = adj @ x
    b0a  = b0 @ W12[C:]
    pre  = x @ Wx + (adj @ x) @ Wa + deg ⊗ b0a + (b1 @ W2 + b2)
    ret  = x + gelu(pre) @ Wo + bo
where deg = adj.sum(1).  The device stores x channel-major as
xtbo = x + bo (bf16); its Wx matmul then over-adds bo @ Wx, which is
subtracted from the gelu bias; the final residual add of xtbo yields
x + bo + gelu(pre) @ Wo directly.

Device work per core (node partition, NS=512 rows, SPMD, no
collectives): sT = (adj @ x_r)^T via fp8 e4m3 DoubleRow matmuls — x_r
tiles are the stationary operand interleaved [k,2,c], this core's
adjacency columns stream as the moving operand [k,2,r], contracting 256
nodes per instruction at 0.5 cycles/row.  The adjacency 0/1 values are
exact in fp8; only x is quantized (rel err ~1.2e-2 vs the 2e-2 gate).
The tiny fused MLP stays bf16 and is interleaved into the staggered
accumulator finish.  DMAs are batched 4 k-chunks per instruction: the
~625ns fixed HWDGE dispatch cost per DMA is the bottleneck at higher
instruction counts (it exceeds the transfer time of these sizes).
"""

import numpy as np
import ml_dtypes

import bass_rust
import concourse.bass as bass
import concourse.mybir as mybir
import concourse.tile as tile
from concourse.bass_utils import run_bass_kernel_spmd

B, N, C, E = 8, 4096, 128, 131072
NCORES = 8
NS = N // NCORES          # 512 output rows per core
KC = N // 128             # 32 k-chunks over the contraction dim
KP = KC // 2              # 16 k-pairs (DoubleRow contracts 256 at a time)
DMA_BLOCKS = [2, 2] + [4] * 7   # k-chunks per DMA block (small first: PE start)
COLS = B * C              # 1024 columns of x_r  (b-major, c-minor)
RCOLS = B * NS            # 4096 columns of transposed row-space tiles

F32 = mybir.dt.float32
BF16 = mybir.dt.bfloat16
FP8 = mybir.dt.float8e4
BF16_NP = ml_dtypes.bfloat16
FP8_NP = ml_dtypes.float8_e4m3

# SW-interleaved DoubleRow weights: the host lays the two stationary
# k-groups out as reversed A/B column pairs so LDWEIGHTS reads them
# contiguously (plain DoubleRow's HW interleave makes the weight load
# ~2x slower, and one LDWEIGHTS is emitted per matmul).
SWI = True


def _split_multiwaits(nc, max_waits=1):
    """Walrus (CoreV3) refuses instructions with more than one sync wait.
    Tile's tail drain can carry several; hoist the extras onto preceding
    single-wait EventSemaphore instructions on the same engine."""
    for blk in nc.m.functions[0].blocks:
        new_list = []
        for ins in blk.instructions:
            si = ins.sync_info
            if si is not None and si.on_wait and len(si.on_wait) > max_waits:
                waits = list(si.on_wait)
                extra, keep = waits[:-max_waits], waits[-max_waits:]
                for i, w in enumerate(extra):
                    ev = mybir.InstEventSemaphore(
                        name=f"{ins.name}_wsplit{i}",
                        engine=ins.engine,
                        ins=[],
                        outs=[],
                        sync_info=bass_rust.SyncInfo(on_wait=[w], on_update=[]),
                    )
                    new_list.append(ev)
                si.on_wait = keep
            new_list.append(ins)
        blk.instructions[:] = new_list


def build_bass(niter=1, stage="full", kb=2, rank1=True, tailk=4, split_waits=True, unroll=1):
    """Build the SPMD program.  niter>1 wraps the whole body in a Tile
    For_i loop — used only for hardware timing (amortizes the very large
    axon dispatch overhead); the graded kernel uses niter=1.
    stage: "full" | "s_only" (timing experiments)."""
    nc = bass.Bass()

    if SWI:
        # interleaved pair layout: row = kp*128 + partition, 2048 cols
        xr_d = nc.dram_tensor("xr", [N // 2, 2 * COLS], FP8, kind="ExternalInput")
    else:
        xr_d = nc.dram_tensor("xr", [N, COLS], FP8, kind="ExternalInput")
    adjT_d = nc.dram_tensor("adjT", [N, NS], FP8, kind="ExternalInput")
    xtbo_d = nc.dram_tensor("xtbo", [C, RCOLS], BF16, kind="ExternalInput")
    wxa_d = nc.dram_tensor("wxa", [C, 3 * C], BF16, kind="ExternalInput")
    vec_d = nc.dram_tensor("vec", [1, NS + C], BF16, kind="ExternalInput")
    b12_d = nc.dram_tensor("b12", [C, 1], F32, kind="ExternalInput")
    out_d = nc.dram_tensor("out", [C, RCOLS], BF16, kind="ExternalOutput")

    # [128, KC, COLS] (plain) or [128, KP, 2*COLS] (swi)
    xr_v = xr_d.rearrange("(k p) c -> p k c", p=128)
    adjT_v = adjT_d.rearrange("(k p) c -> p k c", p=128)  # [128, KC, NS]

    with tile.TileContext(nc) as tc:
        with (
            tc.tile_pool(name="const", bufs=1) as const,
            tc.tile_pool(name="big", bufs=1) as big,
        ):

            def body(_iv=0):
                # ---- resident inputs -------------------------------------
                # k-chunk streams first: the s-stage matmuls chase these.
                # Batched DB chunks per DMA: HWDGE dispatch is ~625ns fixed
                # per instruction, so per-chunk DMAs serialize on dispatch.
                if SWI:
                    xr_sb = big.tile([128, KP, 2 * COLS], FP8)
                else:
                    xr_sb = big.tile([128, KC, COLS], FP8)
                adjT_sb = big.tile([128, KC, NS], FP8)
                xtbo_sb = big.tile([C, RCOLS], BF16)
                wxa_sb = const.tile([C, 3 * C], BF16)
                vec_sb = const.tile([1, NS + C], BF16)
                b12_sb = const.tile([C, 1], F32)
                k0 = 0
                for nch in DMA_BLOCKS:
                    ks = slice(k0, k0 + nch)
                    kps = slice(k0 // 2, (k0 + nch) // 2)
                    k0 += nch
                    # xr first: it is the LDWEIGHTS (stationary) operand
                    nc.sync.dma_start(out=xr_sb[:, kps if SWI else ks, :],
                                      in_=xr_v[:, kps if SWI else ks, :])
                    nc.sync.dma_start(out=adjT_sb[:, ks, :], in_=adjT_v[:, ks, :])
                nc.sync.dma_start(out=xtbo_sb[:], in_=xtbo_d[:])
                nc.sync.dma_start(out=wxa_sb[:], in_=wxa_d[:])
                nc.sync.dma_start(out=vec_sb[:], in_=vec_d[:])
                nc.sync.dma_start(out=b12_sb[:], in_=b12_d[:])

                # ---- sT = (adj @ x_r)^T computed directly: xr tiles are
                # the stationary operand, adjT rows stream as the moving
                # operand, so accumulator bc = batch bc's [c, rows] block of
                # sT.  k-outer over all 8 PSUM banks keeps PE overlapped
                # with the input DMA stream from k=0.  The fused MLP for
                # batch bc-1 is interleaved into accumulator bc's staggered
                # finish so PE never waits on the PSUM pool transition. ----
                sT_sb = big.tile([C, RCOLS], BF16)
                gelu_sb = big.tile([C, RCOLS], BF16)
                res_sb = big.tile([C, RCOLS], BF16)
                with tc.tile_pool(name="psum", bufs=8, space="PSUM") as psum:
                    ps = [
                        psum.tile([128, NS], F32, tag="ps", name=f"sT_acc_{bc}")
                        for bc in range(B)
                    ]
                    TAILK = tailk  # last k-pairs per-acc so stops stagger
                    KB = kb    # k-block: consecutive matmuls per PSUM bank
                    kblocks = [
                        range(k0, min(k0 + KB, KP - TAILK))
                        for k0 in range(0, KP - TAILK, KB)
                    ]
                    def s_lhsT(kp, bc):
                        if SWI:
                            return xr_sb[:, kp, bc * 256:(bc + 1) * 256].rearrange(
                                "p (two m) -> p two m", two=2)
                        return xr_sb[:, 2 * kp:2 * kp + 2, bc * 128:(bc + 1) * 128]

                    S_MODE = (mybir.MatmulPerfMode.DoubleRowSwInterleave if SWI
                              else mybir.MatmulPerfMode.DoubleRow)

                    for kblk in kblocks:
                        for bc in range(B):
                            for kp in kblk:
                                nc.tensor.matmul(
                                    ps[bc],
                                    s_lhsT(kp, bc),
                                    adjT_sb[:, 2 * kp:2 * kp + 2, :],
                                    start=(kp == 0),
                                    stop=False,
                                    perf_mode=S_MODE,
                                )

                    # mlp split in two phases, pipelined one tail-round
                    # apart so the gelu (ACT) latency of batch b hides
                    # under the next accumulator's tail matmuls.
                    pps = {}

                    def mlp_pre(b, c0=0, c1=NS):
                        cols = slice(b * NS + c0, b * NS + c1)
                        if (b, c0) not in pps:
                            pps[(b, c0)] = psum.tile([128, c1 - c0], F32, tag="ps", name=f"pre_{b}_{c0}")
                        pp = pps[(b, c0)]
                        nc.tensor.matmul(pp, wxa_sb[:, 0:C], xtbo_sb[:, cols], start=True, stop=False)
                        if rank1:
                            nc.tensor.matmul(pp, wxa_sb[:, C:2 * C], sT_sb[:, cols], start=False, stop=False)
                            nc.tensor.matmul(pp, vec_sb[:, NS:NS + C], vec_sb[:, c0:c1], start=False, stop=True)
                        else:
                            nc.tensor.matmul(pp, wxa_sb[:, C:2 * C], sT_sb[:, cols], start=False, stop=True)
                        nc.scalar.activation(
                            out=gelu_sb[:, cols], in_=pp[:],
                            func=mybir.ActivationFunctionType.Gelu,
                            bias=b12_sb[:, 0:1], scale=1.0,
                        )

                    def mlp_post(b, c0=0, c1=NS):
                        cols = slice(b * NS + c0, b * NS + c1)
                        po = psum.tile([128, c1 - c0], F32, tag="ps", name=f"out_{b}_{c0}")
                        nc.tensor.matmul(po, wxa_sb[:, 2 * C:3 * C], gelu_sb[:, cols], start=True, stop=True)
                        nc.vector.tensor_add(out=res_sb[:, cols], in0=po[:], in1=xtbo_sb[:, cols])
                        nc.sync.dma_start(out=out_d[:, cols], in_=res_sb[:, cols])

                    for bc in range(B):
                        for kp in range(KP - TAILK, KP):
                            nc.tensor.matmul(
                                ps[bc],
                                s_lhsT(kp, bc),
                                adjT_sb[:, 2 * kp:2 * kp + 2, :],
                                start=False,
                                stop=(kp == KP - 1),
                                perf_mode=S_MODE,
                            )
                        # evacs stay off ACT (it runs the gelus)
                        dst = sT_sb[:, bc * NS:(bc + 1) * NS]
                        nc.vector.tensor_copy(out=dst, in_=ps[bc])
                        if stage == "full":
                            if bc >= 1:
                                mlp_pre(bc - 1)   # evac(bc-1) already landed
                            if bc >= 2:
                                mlp_post(bc - 2)  # gelu(bc-2) already landed
                    if stage == "full":
                        # drain: batch B-1 split column-wise so its gelu
                        # pipelines against batch B-2's wo/add/store
                        mlp_post(B - 2)
                        mlp_pre(B - 1, 0, NS // 2)
                        mlp_pre(B - 1, NS // 2, NS)
                        mlp_post(B - 1, 0, NS // 2)
                        mlp_post(B - 1, NS // 2, NS)
                    else:
                        nc.sync.dma_start(
                            out=out_d[:, 0:NS],
                            in_=sT_sb[:, 0:NS],
                        )

            if niter == 1:
                for _ in range(unroll):
                    body()
            else:
                with tc.For_i(0, niter, 1, hint_engines=(mybir.EngineType.PE,)):
                    for _ in range(unroll):
                        body()

    if split_waits:
        _split_multiwaits(nc)
    return nc


def host_prep(x, edge_index, W0, b0, W1, b1, W2, b2, Wo, bo):
    """Fold weights, build the dense adjacency, lay out per-core inputs."""
    x = np.asarray(x, np.float32)
    ei = np.asarray(edge_index, np.int64)
    W0, b0, W1, b1, W2, b2, Wo, bo = (
        np.asarray(a, np.float32) for a in (W0, b0, W1, b1, W2, b2, Wo, bo)
    )

    # dense symmetric adjacency with set-semantics dedup, zero diagonal
    k1 = ei[0] * N + ei[1]
    k2 = ei[1] * N + ei[0]
    keys = np.unique(np.concatenate([k1, k2]))
    rows = keys // N
    cols = keys % N
    off_diag = rows != cols
    keys, rows = keys[off_diag], rows[off_diag]
    adj = np.zeros(N * N, np.uint8)
    adj[keys] = 0x38  # fp8 e4m3 1.0 bit pattern
    adj = adj.reshape(N, N).view(FP8_NP)
    deg = np.bincount(rows, minlength=N).astype(np.float32)

    # folded weights
    W12 = W1 @ W2                      # [2C, C]
    Wx = W12[:C]
    W12a = W12[C:]
    Wa = W0 @ W12a
    b0a = b0 @ W12a                    # [C]
    # xtbo = x + bo is the Wx-matmul moving operand; subtract the extra
    # bo @ Wx it contributes from the pre-gelu bias
    b12 = (b1 @ W2 + b2 - bo @ Wx).reshape(C, 1).astype(np.float32)
    wxa = np.concatenate([Wx, Wa, Wo], axis=1).astype(BF16_NP)   # [C, 3C]

    xr = np.ascontiguousarray(
        x.transpose(1, 0, 2).reshape(N, B * C)).astype(FP8_NP)   # [N,(b,c)]
    if SWI:
        # per k-pair: columns of the two chunks interleaved pairwise in
        # reversed order: [A_127, B_127, A_126, B_126, ...] per batch block
        KPC = KP * 2
        xr4 = xr.reshape(KP, 2, 128, B, C)        # [kp, two, part, b, c]
        inter = np.empty((KP, 128, B, C, 2), FP8_NP)
        inter[..., 0] = xr4[:, 0].transpose(0, 1, 2, 3)[..., ::-1]
        inter[..., 1] = xr4[:, 1].transpose(0, 1, 2, 3)[..., ::-1]
        xr = np.ascontiguousarray(inter.reshape(KP * 128, 2 * B * C))
    xt = x.transpose(2, 0, 1)                                     # [C,B,N] f32

    in_maps = []
    for c in range(NCORES):
        rs = slice(c * NS, (c + 1) * NS)
        xt_c = np.ascontiguousarray(xt[:, :, rs]).reshape(C, RCOLS)
        vec = np.concatenate([deg[rs], b0a]).reshape(1, NS + C)
        in_maps.append({
            "xr": xr,
            "adjT": np.ascontiguousarray(adj[:, rs]),
            "xtbo": (xt_c + bo[:, None]).astype(BF16_NP),
            "wxa": wxa,
            "vec": vec.astype(BF16_NP),
            "b12": b12,
        })
    return in_maps


def assemble_output(results):
    out = np.empty((B, N, C), np.float32)
    for c in range(NCORES):
        r = results[c]["out"].astype(np.float32)   # [C, (b, row)] bf16
        out[:, c * NS:(c + 1) * NS, :] = r.reshape(C, B, NS).transpose(1, 2, 0)
    return out


_NC_CACHE = []


def kernel(x, edge_index, W0, b0, W1, b1, W2, b2, Wo, bo):
    in_maps = host_prep(x, edge_index, W0, b0, W1, b1, W2, b2, Wo, bo)
    if not _NC_CACHE:
        _NC_CACHE.append(build_bass())
    nc = _NC_CACHE[0]
    res = run_bass_kernel_spmd(nc, in_maps, list(range(NCORES)))
    return assemble_output(res.results)


# revision 31
# speedup vs baseline: 1.0444x; 1.0444x over previous
"""GCNConv-style GNN layer on 8 Trainium2 NeuronCores (Bass/Tile).

Reference computation (B=8, N=4096, C=128, E=131072):
    adj  = symmetric 0/1 adjacency from edge_index, zero diagonal
    h    = x @ W0 + b0
    agg  = adj @ h            (per batch)
    out  = (cat[x, agg] @ W1 + b1) @ W2 + b2
    out  = gelu(out) @ Wo + bo
    ret  = x + out

Algebraic refactor (fold all linear maps before the single GELU on the
host at O(C^2) cost):
    W12  = W1 @ W2                  [2C, C]
    Wx   = W12[:C]                  x-path weight
    Wa   = W0 @ W12[C:]             agg-path weight applied to s = adj @ x
    b0a  = b0 @ W12[C:]
    pre  = x @ Wx + (adj @ x) @ Wa + deg ⊗ b0a + (b1 @ W2 + b2)
    ret  = x + gelu(pre) @ Wo + bo
where deg = adj.sum(1).  The device stores x channel-major as
xtbo = x + bo (bf16); its Wx matmul then over-adds bo @ Wx, which is
subtracted from the gelu bias; the final residual add of xtbo yields
x + bo + gelu(pre) @ Wo directly.

Device work per core (node partition, NS=512 rows, SPMD, no
collectives): sT = (adj @ x_r)^T via fp8 e4m3 DoubleRow matmuls — x_r
tiles are the stationary operand interleaved [k,2,c], this core's
adjacency columns stream as the moving operand [k,2,r], contracting 256
nodes per instruction at 0.5 cycles/row.  The adjacency 0/1 values are
exact in fp8; only x is quantized (rel err ~1.2e-2 vs the 2e-2 gate).
The tiny fused MLP stays bf16 and is interleaved into the staggered
accumulator finish.  DMAs are batched 4 k-chunks per instruction: the
~625ns fixed HWDGE dispatch cost per DMA is the bottleneck at higher
instruction counts (it exceeds the transfer time of these sizes).
"""

import numpy as np
import ml_dtypes

import bass_rust
import concourse.bass as bass
import concourse.mybir as mybir
import concourse.tile as tile
from concourse.bass_utils import run_bass_kernel_spmd

B, N, C, E = 8, 4096, 128, 131072
NCORES = 8
NS = N // NCORES          # 512 output rows per core
KC = N // 128             # 32 k-chunks over the contraction dim
KP = KC // 2              # 16 k-pairs (DoubleRow contracts 256 at a time)
DMA_BLOCKS = [2, 2] + [4] * 7   # k-chunks per DMA block (small first: PE start)
COLS = B * C              # 1024 columns of x_r  (b-major, c-minor)
RCOLS = B * NS            # 4096 columns of transposed row-space tiles

F32 = mybir.dt.float32
BF16 = mybir.dt.bfloat16
FP8 = mybir.dt.float8e4
BF16_NP = ml_dtypes.bfloat16
FP8_NP = ml_dtypes.float8_e4m3

# SW-interleaved DoubleRow weights (host lays the two stationary
# k-groups out as reversed A/B column pairs).  HW-measured 72.1us vs
# 47.0us for plain DoubleRow — walrus/HW handles the SwInterleave
# weight read badly.  Keep False.
SWI = False


def _split_multiwaits(nc, max_waits=1):
    """Walrus (CoreV3) refuses instructions with more than one sync wait.
    Tile's tail drain can carry several; hoist the extras onto preceding
    single-wait EventSemaphore instructions on the same engine."""
    for blk in nc.m.functions[0].blocks:
        new_list = []
        for ins in blk.instructions:
            si = ins.sync_info
            if si is not None and si.on_wait and len(si.on_wait) > max_waits:
                waits = list(si.on_wait)
                extra, keep = waits[:-max_waits], waits[-max_waits:]
                for i, w in enumerate(extra):
                    ev = mybir.InstEventSemaphore(
                        name=f"{ins.name}_wsplit{i}",
                        engine=ins.engine,
                        ins=[],
                        outs=[],
                        sync_info=bass_rust.SyncInfo(on_wait=[w], on_update=[]),
                    )
                    new_list.append(ev)
                si.on_wait = keep
            new_list.append(ins)
        blk.instructions[:] = new_list


def build_bass(niter=1, stage="full", kb=2, rank1=True, tailk=4, split_waits=True, unroll=1):
    """Build the SPMD program.  niter>1 wraps the whole body in a Tile
    For_i loop — used only for hardware timing (amortizes the very large
    axon dispatch overhead); the graded kernel uses niter=1.
    stage: "full" | "s_only" (timing experiments)."""
    nc = bass.Bass()

    if SWI:
        # interleaved pair layout: row = kp*128 + partition, 2048 cols
        xr_d = nc.dram_tensor("xr", [N // 2, 2 * COLS], FP8, kind="ExternalInput")
    else:
        xr_d = nc.dram_tensor("xr", [N, COLS], FP8, kind="ExternalInput")
    adjT_d = nc.dram_tensor("adjT", [N, NS], FP8, kind="ExternalInput")
    xtbo_d = nc.dram_tensor("xtbo", [C, RCOLS], BF16, kind="ExternalInput")
    wxa_d = nc.dram_tensor("wxa", [C, 3 * C], BF16, kind="ExternalInput")
    vec_d = nc.dram_tensor("vec", [1, NS + C], BF16, kind="ExternalInput")
    b12_d = nc.dram_tensor("b12", [C, 1], F32, kind="ExternalInput")
    out_d = nc.dram_tensor("out", [C, RCOLS], BF16, kind="ExternalOutput")

    # [128, KC, COLS] (plain) or [128, KP, 2*COLS] (swi)
    xr_v = xr_d.rearrange("(k p) c -> p k c", p=128)
    adjT_v = adjT_d.rearrange("(k p) c -> p k c", p=128)  # [128, KC, NS]

    with tile.TileContext(nc) as tc:
        with (
            tc.tile_pool(name="const", bufs=1) as const,
            tc.tile_pool(name="big", bufs=1) as big,
        ):

            def body(_iv=0):
                # ---- resident inputs -------------------------------------
                # k-chunk streams first: the s-stage matmuls chase these.
                # Batched DB chunks per DMA: HWDGE dispatch is ~625ns fixed
                # per instruction, so per-chunk DMAs serialize on dispatch.
                if SWI:
                    xr_sb = big.tile([128, KP, 2 * COLS], FP8)
                else:
                    xr_sb = big.tile([128, KC, COLS], FP8)
                adjT_sb = big.tile([128, KC, NS], FP8)
                xtbo_sb = big.tile([C, RCOLS], BF16)
                wxa_sb = const.tile([C, 3 * C], BF16)
                vec_sb = const.tile([1, NS + C], BF16)
                b12_sb = const.tile([C, 1], F32)
                k0 = 0
                for nch in DMA_BLOCKS:
                    ks = slice(k0, k0 + nch)
                    kps = slice(k0 // 2, (k0 + nch) // 2)
                    k0 += nch
                    # xr first: it is the LDWEIGHTS (stationary) operand
                    nc.sync.dma_start(out=xr_sb[:, kps if SWI else ks, :],
                                      in_=xr_v[:, kps if SWI else ks, :])
                    nc.sync.dma_start(out=adjT_sb[:, ks, :], in_=adjT_v[:, ks, :])
                nc.sync.dma_start(out=xtbo_sb[:], in_=xtbo_d[:])
                nc.sync.dma_start(out=wxa_sb[:], in_=wxa_d[:])
                nc.sync.dma_start(out=vec_sb[:], in_=vec_d[:])
                nc.sync.dma_start(out=b12_sb[:], in_=b12_d[:])

                # ---- sT = (adj @ x_r)^T computed directly: xr tiles are
                # the stationary operand, adjT rows stream as the moving
                # operand, so accumulator bc = batch bc's [c, rows] block of
                # sT.  k-outer over all 8 PSUM banks keeps PE overlapped
                # with the input DMA stream from k=0.  The fused MLP for
                # batch bc-1 is interleaved into accumulator bc's staggered
                # finish so PE never waits on the PSUM pool transition. ----
                sT_sb = big.tile([C, RCOLS], BF16)
                gelu_sb = big.tile([C, RCOLS], BF16)
                res_sb = big.tile([C, RCOLS], BF16)
                with tc.tile_pool(name="psum", bufs=8, space="PSUM") as psum:
                    ps = [
                        psum.tile([128, NS], F32, tag="ps", name=f"sT_acc_{bc}")
                        for bc in range(B)
                    ]
                    TAILK = tailk  # last k-pairs per-acc so stops stagger
                    KB = kb    # k-block: consecutive matmuls per PSUM bank
                    kblocks = [
                        range(k0, min(k0 + KB, KP - TAILK))
                        for k0 in range(0, KP - TAILK, KB)
                    ]
                    def s_lhsT(kp, bc):
                        if SWI:
                            return xr_sb[:, kp, bc * 256:(bc + 1) * 256].rearrange(
                                "p (two m) -> p two m", two=2)
                        return xr_sb[:, 2 * kp:2 * kp + 2, bc * 128:(bc + 1) * 128]

                    S_MODE = (mybir.MatmulPerfMode.DoubleRowSwInterleave if SWI
                              else mybir.MatmulPerfMode.DoubleRow)

                    for kblk in kblocks:
                        for bc in range(B):
                            for kp in kblk:
                                nc.tensor.matmul(
                                    ps[bc],
                                    s_lhsT(kp, bc),
                                    adjT_sb[:, 2 * kp:2 * kp + 2, :],
                                    start=(kp == 0),
                                    stop=False,
                                    perf_mode=S_MODE,
                                )

                    # mlp split in two phases, pipelined one tail-round
                    # apart so the gelu (ACT) latency of batch b hides
                    # under the next accumulator's tail matmuls.
                    pps = {}

                    def mlp_pre(b, c0=0, c1=NS):
                        cols = slice(b * NS + c0, b * NS + c1)
                        if (b, c0) not in pps:
                            pps[(b, c0)] = psum.tile([128, c1 - c0], F32, tag="ps", name=f"pre_{b}_{c0}")
                        pp = pps[(b, c0)]
                        nc.tensor.matmul(pp, wxa_sb[:, 0:C], xtbo_sb[:, cols], start=True, stop=False)
                        if rank1:
                            nc.tensor.matmul(pp, wxa_sb[:, C:2 * C], sT_sb[:, cols], start=False, stop=False)
                            nc.tensor.matmul(pp, vec_sb[:, NS:NS + C], vec_sb[:, c0:c1], start=False, stop=True)
                        else:
                            nc.tensor.matmul(pp, wxa_sb[:, C:2 * C], sT_sb[:, cols], start=False, stop=True)
                        nc.scalar.activation(
                            out=gelu_sb[:, cols], in_=pp[:],
                            func=mybir.ActivationFunctionType.Gelu,
                            bias=b12_sb[:, 0:1], scale=1.0,
                        )

                    def mlp_post(b, c0=0, c1=NS):
                        cols = slice(b * NS + c0, b * NS + c1)
                        po = psum.tile([128, c1 - c0], F32, tag="ps", name=f"out_{b}_{c0}")
                        nc.tensor.matmul(po, wxa_sb[:, 2 * C:3 * C], gelu_sb[:, cols], start=True, stop=True)
                        nc.vector.tensor_add(out=res_sb[:, cols], in0=po[:], in1=xtbo_sb[:, cols])
                        nc.sync.dma_start(out=out_d[:, cols], in_=res_sb[:, cols])

                    for bc in range(B):
                        for kp in range(KP - TAILK, KP):
                            nc.tensor.matmul(
                                ps[bc],
                                s_lhsT(kp, bc),
                                adjT_sb[:, 2 * kp:2 * kp + 2, :],
                                start=False,
                                stop=(kp == KP - 1),
                                perf_mode=S_MODE,
                            )
                        # evacs stay off ACT (it runs the gelus)
                        dst = sT_sb[:, bc * NS:(bc + 1) * NS]
                        nc.vector.tensor_copy(out=dst, in_=ps[bc])
                        if stage == "full":
                            if bc >= 1:
                                mlp_pre(bc - 1)   # evac(bc-1) already landed
                            if bc >= 2:
                                mlp_post(bc - 2)  # gelu(bc-2) already landed
                    if stage == "full":
                        # drain: batch B-1 split column-wise so its gelu
                        # pipelines against batch B-2's wo/add/store
                        mlp_post(B - 2)
                        q = NS // 4
                        mlp_pre(B - 1, 0, q)
                        mlp_pre(B - 1, q, 2 * q)
                        mlp_post(B - 1, 0, q)
                        mlp_pre(B - 1, 2 * q, 3 * q)
                        mlp_post(B - 1, q, 2 * q)
                        mlp_pre(B - 1, 3 * q, NS)
                        mlp_post(B - 1, 2 * q, 3 * q)
                        mlp_post(B - 1, 3 * q, NS)
                    else:
                        nc.sync.dma_start(
                            out=out_d[:, 0:NS],
                            in_=sT_sb[:, 0:NS],
                        )

            if niter == 1:
                for _ in range(unroll):
                    body()
            else:
                with tc.For_i(0, niter, 1, hint_engines=(mybir.EngineType.PE,)):
                    for _ in range(unroll):
                        body()

    if split_waits:
        _split_multiwaits(nc)
    return nc


def host_prep(x, edge_index, W0, b0, W1, b1, W2, b2, Wo, bo):
    """Fold weights, build the dense adjacency, lay out per-core inputs."""
    x = np.asarray(x, np.float32)
    ei = np.asarray(edge_index, np.int64)
    W0, b0, W1, b1, W2, b2, Wo, bo = (
        np.asarray(a, np.float32) for a in (W0, b0, W1, b1, W2, b2, Wo, bo)
    )

    # dense symmetric adjacency with set-semantics dedup, zero diagonal
    k1 = ei[0] * N + ei[1]
    k2 = ei[1] * N + ei[0]
    keys = np.unique(np.concatenate([k1, k2]))
    rows = keys // N
    cols = keys % N
    off_diag = rows != cols
    keys, rows = keys[off_diag], rows[off_diag]
    adj = np.zeros(N * N, np.uint8)
    adj[keys] = 0x38  # fp8 e4m3 1.0 bit pattern
    adj = adj.reshape(N, N).view(FP8_NP)
    deg = np.bincount(rows, minlength=N).astype(np.float32)

    # folded weights
    W12 = W1 @ W2                      # [2C, C]
    Wx = W12[:C]
    W12a = W12[C:]
    Wa = W0 @ W12a
    b0a = b0 @ W12a                    # [C]
    # xtbo = x + bo is the Wx-matmul moving operand; subtract the extra
    # bo @ Wx it contributes from the pre-gelu bias
    b12 = (b1 @ W2 + b2 - bo @ Wx).reshape(C, 1).astype(np.float32)
    wxa = np.concatenate([Wx, Wa, Wo], axis=1).astype(BF16_NP)   # [C, 3C]

    xr = np.ascontiguousarray(
        x.transpose(1, 0, 2).reshape(N, B * C)).astype(FP8_NP)   # [N,(b,c)]
    if SWI:
        # per k-pair: columns of the two chunks interleaved pairwise in
        # reversed order: [A_127, B_127, A_126, B_126, ...] per batch block
        KPC = KP * 2
        xr4 = xr.reshape(KP, 2, 128, B, C)        # [kp, two, part, b, c]
        inter = np.empty((KP, 128, B, C, 2), FP8_NP)
        inter[..., 0] = xr4[:, 0].transpose(0, 1, 2, 3)[..., ::-1]
        inter[..., 1] = xr4[:, 1].transpose(0, 1, 2, 3)[..., ::-1]
        xr = np.ascontiguousarray(inter.reshape(KP * 128, 2 * B * C))
    xt = x.transpose(2, 0, 1)                                     # [C,B,N] f32

    in_maps = []
    for c in range(NCORES):
        rs = slice(c * NS, (c + 1) * NS)
        xt_c = np.ascontiguousarray(xt[:, :, rs]).reshape(C, RCOLS)
        vec = np.concatenate([deg[rs], b0a]).reshape(1, NS + C)
        in_maps.append({
            "xr": xr,
            "adjT": np.ascontiguousarray(adj[:, rs]),
            "xtbo": (xt_c + bo[:, None]).astype(BF16_NP),
            "wxa": wxa,
            "vec": vec.astype(BF16_NP),
            "b12": b12,
        })
    return in_maps


def assemble_output(results):
    out = np.empty((B, N, C), np.float32)
    for c in range(NCORES):
        r = results[c]["out"].astype(np.float32)   # [C, (b, row)] bf16
        out[:, c * NS:(c + 1) * NS, :] = r.reshape(C, B, NS).transpose(1, 2, 0)
    return out


_NC_CACHE = []


def kernel(x, edge_index, W0, b0, W1, b1, W2, b2, Wo, bo):
    in_maps = host_prep(x, edge_index, W0, b0, W1, b1, W2, b2, Wo, bo)
    if not _NC_CACHE:
        _NC_CACHE.append(build_bass())
    nc = _NC_CACHE[0]
    res = run_bass_kernel_spmd(nc, in_maps, list(range(NCORES)))
    return assemble_output(res.results)
